# revision 3
# baseline (speedup 1.0000x reference)
"""Trainium2 Bass kernel for nn_DecoderBlock — v2 (bf16 pipeline).

Data-parallel over batch N=8 -> one batch element per NeuronCore.

Changes vs v1 baseline (894us):
  - All matmul operands bf16 (halved DMA/SBUF; PE speed unchanged at
    1 cyc/row; error budget allows: harness gate is 2e-2).
  - Softmax reciprocal via reciprocal_approx_fast (5x faster DVE op).
  - psum->bf16 casts moved to the Act engine (DVE was near the
    attention critical path).
  - Startup: xt_dec split in k-halves, enc_xt/dec_nat deferred, weight
    DMAs issued in consumption order with deep prefetch (bufs=6).
  - bf16 PE transposes (1 cyc/row) fed by an Act-engine LN re-apply.
  - ca wo_ln block gets FFN-W1 chunks as PE filler; W2 restructured to
    per-(mh, ks) chains so the last LN/DMA tail is ~4us not ~15us.
"""
import sys

sys.path.insert(0, "/opt/trn_rl_repo")

import numpy as np
import ml_dtypes

try:
    from antenv import axon_hooks as _ah  # noqa: F401
except ImportError:
    import types as _types

    _h = _types.ModuleType("antenv.axon_hooks")
    _h._HOOK = None

    def _get_hook():
        if _h._HOOK is None:
            try:
                from trn_agent_boot.trn_boot import _ntff_profile_via_ctypes

                _h._HOOK = _ntff_profile_via_ctypes("/opt/axon/libaxon_pjrt.so")
            except Exception:
                _h._HOOK = None
        return _h._HOOK

    _h.get_axon_ntff_profile_hook = _get_hook
    _h.set_axon_ntff_profile_hook = lambda hook: setattr(_h, "_HOOK", hook)
    sys.modules["antenv.axon_hooks"] = _h

import concourse.bass as bass
import concourse.tile as tile
from concourse import bacc, mybir
from concourse.bass_utils import run_bass_kernel_spmd
from concourse.masks import make_identity

F32 = mybir.dt.float32
BF16 = mybir.dt.bfloat16
AF = mybir.ActivationFunctionType
OP = mybir.AluOpType

P = 128
K = 1024
M = 1024
H = 8
DH = 128
HD = H * DH
FF = 4096
KT = K // P      # 8
MT = M // P      # 8
HT = HD // P     # 8
FT = FF // P     # 32
QW = 512
EPS = 1e-10
ISQ = 1.0 / float(np.sqrt(DH))

N_CORES = 8
BF = ml_dtypes.bfloat16


def _bcast_row_ap(t: bass.AP, width: int) -> bass.AP:
    return bass.AP(tensor=t.tensor, offset=t.offset, ap=[[0, P], [1, width]])


def build_kernel(flags: dict):
    nc = bacc.Bacc("TRN2", target_bir_lowering=False, debug=False,
                   num_devices=N_CORES)
    dram = {}

    def din(name, shape, dt=BF16):
        dram[name] = nc.dram_tensor(name, shape, dt, kind="ExternalInput").ap()

    din("xt_dec", (P, MT * K))
    din("xt_enc", (P, MT * K))
    din("wq_sa", (H, P, MT * DH)); din("wk_sa", (H, P, MT * DH))
    din("wq_ca", (H, P, MT * DH)); din("wk_ca", (H, P, MT * DH))
    din("wv_sa", (MT, P, HD)); din("wv_ca", (MT, P, HD))
    din("wo_sa", (HT, P, M)); din("wo_ca", (HT, P, M))
    din("w1", (FT, P, MT * P)); din("w2", (FT, P, M))
    din("dec_nat", (K, M))
    for nm in ("bq_sa", "bk_sa", "bq_ca", "bk_ca"):
        if flags[nm]:
            din(nm, (DH, H), F32)
    for nm in ("bv_sa", "bv_ca", "bo_sa", "bo_ca", "bf2",
               "g1", "b1", "g2", "b2", "g3", "b3"):
        if flags[nm]:
            din(nm, (M,), F32)
    if flags["bf1"]:
        din("bf1", (P, FT), F32)
    out = nc.dram_tensor("out", (K, M), F32, kind="ExternalOutput").ap()

    with tile.TileContext(nc) as tc:
        _emit(nc, tc, dram, out, flags)
    nc.compile()
    return nc


def _emit(nc, tc, dram, out, flags):
    from contextlib import ExitStack

    with ExitStack() as ctx:
        # ---------- persistent pools ----------
        const = ctx.enter_context(tc.tile_pool(name="const", bufs=1))
        natp = ctx.enter_context(tc.tile_pool(name="natp", bufs=2))
        residp = ctx.enter_context(tc.tile_pool(name="residp", bufs=4))
        statp = ctx.enter_context(tc.tile_pool(name="statp", bufs=4))
        xbfp = ctx.enter_context(tc.tile_pool(name="xbfp", bufs=2))
        xtp = ctx.enter_context(tc.tile_pool(name="xtp", bufs=2))
        ycatp = ctx.enter_context(tc.tile_pool(name="ycatp", bufs=1))
        wop = ctx.enter_context(tc.tile_pool(name="wop", bufs=1))
        ps = ctx.enter_context(tc.tile_pool(name="ps", bufs=6, space="PSUM"))
        dscr = ctx.enter_context(tc.tile_pool(name="dscr", bufs=1,
                                              space="DRAM"))

        ident = const.tile([P, P], BF16, name="ident")
        make_identity(nc, ident)
        ones_t = const.tile([P, P], BF16, name="ones_t")
        nc.vector.memset(ones_t, 1.0)
        eps_t = const.tile([P, 1], F32, name="eps_t")
        nc.vector.memset(eps_t, EPS)

        bias_tiles = {}
        for nm in ("bq_sa", "bk_sa", "bq_ca", "bk_ca"):
            if flags[nm]:
                t = const.tile([P, H], F32, name=nm + "_t")
                nc.sync.dma_start(out=t, in_=dram[nm])
                bias_tiles[nm] = t
        if flags["bf1"]:
            t = const.tile([P, FT], F32, name="bf1_t")
            nc.sync.dma_start(out=t, in_=dram["bf1"])
            bias_tiles["bf1"] = t
        for nm in ("bv_sa", "bv_ca", "bo_sa", "bo_ca", "bf2",
                   "g1", "b1", "g2", "b2", "g3", "b3"):
            if flags[nm]:
                t = const.tile([P, M], F32, name=nm + "_t")
                nc.sync.dma_start(out=t, in_=_bcast_row_ap(dram[nm], M))
                bias_tiles[nm] = t

        x1_store = dscr.tile([K, M], BF16, name="x1_store")
        x2_store = dscr.tile([K, M], BF16, name="x2_store")

        def new_xt(name):
            return xtp.tile([P, MT, K], BF16, name=name, tag="xt_slot")

        # ---------------- building blocks ----------------

        def psum_to_bf16(dst, src_ps, bias_ap=None):
            """Act-engine copy psum f32 -> bf16 tile (optional col bias)."""
            if bias_ap is not None:
                nc.scalar.activation(dst, src_ps, AF.Identity, bias=bias_ap)
            else:
                nc.scalar.activation(dst, src_ps, AF.Identity)

        def qk_w_tile(qkp, w_name, h):
            w = qkp.tile([P, MT, DH], BF16, name=f"{w_name}_{h}", tag="wqk",
                         bufs=6)
            nc.sync.dma_start(out=w, in_=dram[w_name][h].rearrange(
                "p (mt d) -> p mt d", mt=MT))
            return w

        def qk_head_proj(qkp, qhp, src_xt, w_name, b_name, h, nm, prew=None):
            """returns qh tile [P(dh), K] bf16 for head h."""
            w = prew if prew is not None else qk_w_tile(qkp, w_name, h)
            qh = qhp.tile([P, K], BF16, name=f"{nm}_{h}", tag=nm, bufs=2)
            for kh2 in range(2):
                pq = ps.tile([P, 512], F32, name=f"pq_{nm}_{h}_{kh2}",
                             tag="ps")
                for mt in range(MT):
                    nc.tensor.matmul(
                        pq, w[:, mt, :],
                        src_xt[:, mt, kh2 * 512:(kh2 + 1) * 512],
                        start=(mt == 0), stop=(mt == MT - 1))
                d = qh[:, kh2 * 512:(kh2 + 1) * 512]
                if b_name is not None and flags[b_name]:
                    psum_to_bf16(d, pq, bias_tiles[b_name][:, h:h + 1])
                else:
                    psum_to_bf16(d, pq)
            return qh

        def v_chunk_cbs(wvp, src_xt, wv_name, bv_name, vcat):
            """Returns (loads, chunks): 2 weight-load cbs (one per 512-wide
            hd group) and 16 chunk cbs (8-matmul psum chain each). Loads can
            be emitted early so chunk chains never wait on their DMA."""
            state = {}

            def make_load(g):
                def load():
                    tiles = []
                    for mt in range(MT):
                        w = wvp.tile([P, 512], BF16, name=f"wv{g}_{mt}",
                                     tag="wvh", bufs=8)
                        nc.sync.dma_start(
                            out=w,
                            in_=dram[wv_name][mt, :, g * 512:(g + 1) * 512])
                        tiles.append(w)
                    state[g] = tiles
                return load

            def make_chunk(g, kt):
                def chunk():
                    wvts = state[g]
                    pv = ps.tile([P, 512], F32, name=f"psv{g}_{kt}", tag="ps")
                    for mt in range(MT):
                        nc.tensor.matmul(
                            pv, src_xt[:, mt, kt * P:(kt + 1) * P],
                            wvts[mt], start=(mt == 0), stop=(mt == MT - 1))
                    dst = vcat[:, kt, g * 512:(g + 1) * 512]
                    if flags[bv_name]:
                        nc.vector.scalar_tensor_tensor(
                            out=dst, in0=pv, scalar=1.0,
                            in1=bias_tiles[bv_name][:, g * 512:(g + 1) * 512],
                            op0=OP.bypass, op1=OP.add)
                    else:
                        psum_to_bf16(dst, pv)
                return chunk

            loads = [make_load(g) for g in range(2)]
            chunks = [make_chunk(g, kt) for g in range(2) for kt in range(KT)]
            return loads, chunks

        def sda(attp, h, qh, kh, vcat, ycat, pending):
            """scores -> exp(bf16) -> denom -> 1/x -> AV -> ycat.

            The softmax denominator's kt-accumulation runs as a DVE bf16
            add tree, so the partition-sum ones-matmul is a single 512-row
            chain instead of 8 (frees ~48us of PE). bf16 rounding in the
            tree averages out over the 128-partition matmul sum. The
            psd/recip/ycat finish of each (h,q) is DEFERRED one scores
            block so the in-order PE never waits on the DVE tree."""
            for q in range(2):
                expq = attp.tile([P, KT, QW], BF16, name=f"ex_{h}_{q}",
                                 tag="expq", bufs=2)
                for kt in range(KT):
                    pss = ps.tile([P, QW], F32, name=f"pss{h}_{q}_{kt}",
                                  tag="ps")
                    nc.tensor.matmul(
                        pss, kh[:, kt * P:(kt + 1) * P],
                        qh[:, q * QW:(q + 1) * QW], start=True, stop=True)
                    nc.scalar.activation(expq[:, kt, :], pss, AF.Exp,
                                         scale=ISQ)
                tr = [attp.tile([P, QW], BF16, name=f"tr{i}_{h}_{q}",
                                tag=f"tr{i}", bufs=2) for i in range(4)]
                nc.vector.tensor_add(tr[0], expq[:, 0, :], expq[:, 1, :])
                nc.vector.tensor_add(tr[1], expq[:, 2, :], expq[:, 3, :])
                nc.vector.tensor_add(tr[2], expq[:, 4, :], expq[:, 5, :])
                nc.vector.tensor_add(tr[3], expq[:, 6, :], expq[:, 7, :])
                nc.vector.tensor_add(tr[0], tr[0], tr[1])
                nc.vector.tensor_add(tr[2], tr[2], tr[3])
                nc.vector.tensor_add(tr[0], tr[0], tr[2])
                if pending is not None:
                    pending()
                psy = ps.tile([P, QW], F32, name=f"psy{h}_{q}", tag="psy",
                              bufs=2)
                for kt in range(KT):
                    nc.tensor.matmul(
                        psy, vcat[:, kt, h * DH:(h + 1) * DH],
                        expq[:, kt, :], start=(kt == 0), stop=(kt == KT - 1))

                def make_finish(h=h, q=q, tr0=tr[0], psy=psy):
                    def finish():
                        psd = ps.tile([P, QW], F32, name=f"psd{h}_{q}",
                                      tag="ps")
                        nc.tensor.matmul(psd, ones_t, tr0, start=True,
                                         stop=True)
                        recip = attp.tile([P, QW], F32, name=f"rc_{h}_{q}",
                                          tag="recip", bufs=2)
                        nc.vector.reciprocal_approx_fast(out=recip, in_=psd)
                        nc.vector.tensor_mul(
                            ycat[:, h, q * QW:(q + 1) * QW], psy, recip)
                    return finish

                pending = make_finish()
            return pending

        def ln_tail(z, kt, g_name, b_name, store_dram, to_out, want_bf16,
                    stats=None, done_sg=0):
            """LN(z); store f32 to dram; optionally emit bf16 copy for the
            transposes. Returns the bf16 tile (or None). `stats`/`done_sg`
            let the caller pre-compute the first half's bn_stats early."""
            if stats is None:
                stats = statp.tile([P, 2, 6], F32, name=f"st{kt}",
                                   tag="stats")
            for sg in range(done_sg, 2):
                nc.vector.bn_stats(out=stats[:, sg, :],
                                   in_=z[:, sg * 512:(sg + 1) * 512])
            mv = statp.tile([P, 2], F32, name=f"mv{kt}", tag="mv")
            nc.vector.bn_aggr(out=mv, in_=stats)
            std = statp.tile([P, 1], F32, name=f"sd{kt}", tag="std")
            nc.scalar.activation(std, mv[:, 1:2], AF.Sqrt, bias=eps_t)
            inv = statp.tile([P, 1], F32, name=f"iv{kt}", tag="inv")
            nc.vector.reciprocal(inv, std)
            gb_flags = flags[g_name] or flags[b_name]
            if to_out and not gb_flags:
                # split apply+DMA in halves so the last tile's store
                # overlaps its own LN apply (shorter kernel tail)
                x = natp.tile([P, M], F32, name=f"x{kt}", tag="x", bufs=2)
                for sg in range(2):
                    sl = slice(sg * 512, (sg + 1) * 512)
                    nc.vector.tensor_scalar(
                        out=x[:, sl], in0=z[:, sl], scalar1=mv[:, 0:1],
                        scalar2=inv, op0=OP.subtract, op1=OP.mult)
                    nc.sync.dma_start(out=out[kt * P:(kt + 1) * P, sl],
                                      in_=x[:, sl])
                return None
            if not gb_flags:
                # store blocks (want_bf16): the bf16 LN re-apply on the Act
                # engine IS the stored value — no f32 apply, half the DMA
                nmi = statp.tile([P, 1], F32, name=f"nmi{kt}", tag="nmi")
                nc.vector.scalar_tensor_tensor(
                    out=nmi, in0=mv[:, 0:1], scalar=-1.0, in1=inv,
                    op0=OP.mult, op1=OP.mult)
                xb = xbfp.tile([P, M], BF16, name=f"xb{kt}", tag="xb",
                               bufs=2)
                nc.scalar.activation(xb, z, AF.Identity, scale=inv, bias=nmi)
                nc.sync.dma_start(out=store_dram[kt * P:(kt + 1) * P, :],
                                  in_=xb)
                return xb
            x = natp.tile([P, M], F32, name=f"x{kt}", tag="x", bufs=2)
            nc.vector.tensor_scalar(out=x, in0=z, scalar1=mv[:, 0:1],
                                    scalar2=inv, op0=OP.subtract, op1=OP.mult)
            if flags[g_name]:
                nc.vector.tensor_mul(x, x, bias_tiles[g_name])
            if flags[b_name]:
                nc.vector.tensor_add(x, x, bias_tiles[b_name])
            if to_out:
                nc.sync.dma_start(out=out[kt * P:(kt + 1) * P, :], in_=x)
                return None
            xb = xbfp.tile([P, M], BF16, name=f"xb{kt}", tag="xb", bufs=2)
            nc.scalar.activation(xb, x, AF.Identity)
            nc.sync.dma_start(out=store_dram[kt * P:(kt + 1) * P, :],
                              in_=xb)
            return xb

        def transpose_into(xb, kt, xt_new):
            for mt in range(MT):
                pt = ps.tile([P, P], BF16, name=f"ptr{kt}_{mt}", tag="ps")
                nc.tensor.transpose(pt, xb[:, mt * P:(mt + 1) * P], ident)
                nc.vector.tensor_copy(xt_new[:, mt, kt * P:(kt + 1) * P], pt)

        def load_wo(wo_name):
            wots = []
            for ht in range(HT):
                w = wop.tile([P, M], BF16, name=f"{wo_name}{ht}", tag="wo8",
                             bufs=8)
                nc.sync.dma_start(out=w, in_=dram[wo_name][ht])
                wots.append(w)
            return wots

        def wo_ln_block(ycat, wots, bo_name, resid_dram, g_name, b_name,
                        store_dram, xt_new, filler=(), fill_per_kt=3,
                        fill_start=0):
            """a = ycat @ Wo^T; z = resid + a; LN; bf16-transpose into xt_new.
            filler: PE-work callbacks interleaved to keep the PE fed."""
            filler = list(filler)
            xb_prev = None
            resids = []
            for kt in range(4):
                r = residp.tile([P, M], BF16, name=f"rs{kt}", tag="resid")
                nc.sync.dma_start(
                    out=r, in_=resid_dram[kt * P:(kt + 1) * P, :])
                resids.append(r)
            for kt in range(KT):
                resid = resids[kt]
                if kt + 4 < KT:
                    r = residp.tile([P, M], BF16, name=f"rs{kt + 4}",
                                    tag="resid")
                    nc.sync.dma_start(
                        out=r,
                        in_=resid_dram[(kt + 4) * P:(kt + 5) * P, :])
                    resids.append(r)
                z = natp.tile([P, M], F32, name=f"z{kt}", tag="z")
                for mh in range(2):
                    pa = ps.tile([P, 512], F32, name=f"pa{kt}_{mh}", tag="ps")
                    for ht in range(HT):
                        nc.tensor.matmul(
                            pa, ycat[:, ht, kt * P:(kt + 1) * P],
                            wots[ht][:, mh * 512:(mh + 1) * 512],
                            start=(ht == 0), stop=(ht == HT - 1))
                    sl = slice(mh * 512, (mh + 1) * 512)
                    if flags[bo_name]:
                        nc.vector.scalar_tensor_tensor(
                            out=z[:, sl], in0=pa, scalar=1.0,
                            in1=bias_tiles[bo_name][:, sl],
                            op0=OP.bypass, op1=OP.add)
                        nc.vector.tensor_add(z[:, sl], z[:, sl], resid[:, sl])
                    else:
                        nc.vector.tensor_add(z[:, sl], pa, resid[:, sl])
                xb = ln_tail(z, kt, g_name, b_name, store_dram, False, True)
                if kt >= fill_start:
                    for _ in range(fill_per_kt):
                        if filler:
                            filler.pop(0)()
                if xb_prev is not None:
                    transpose_into(xb_prev, kt - 1, xt_new)
                xb_prev = xb
            # leftover fillers run before the last transpose so the PE
            # isn't stalled on the final LN chain
            for cb in filler:
                cb()
            transpose_into(xb_prev, KT - 1, xt_new)

        # ================= SA attention =================
        att_ctx = tc.tile_pool(name="attp", bufs=1)
        attp = att_ctx.__enter__()
        vc_ctx = tc.tile_pool(name="vcp", bufs=1)
        vcp = vc_ctx.__enter__()

        # startup order: the first q-proj chain reads xt mt-by-mt, so land
        # mt0-1 of the first k-half plus wq_h0 first (~0.5MB) and stream
        # the rest behind them.
        xt = new_xt("decT")
        xt_ap = dram["xt_dec"].rearrange("p (mt k) -> p mt k", mt=MT)
        nc.sync.dma_start(out=xt[:, 0:2, 0:512], in_=xt_ap[:, 0:2, 0:512])
        prew_q0 = qk_w_tile(vcp, "wq_sa", 0)
        nc.sync.dma_start(out=xt[:, 2:MT, 0:512], in_=xt_ap[:, 2:MT, 0:512])
        prew_k0 = qk_w_tile(vcp, "wk_sa", 0)
        nc.sync.dma_start(out=xt[:, :, 512:1024], in_=xt_ap[:, :, 512:1024])

        vcat_sa = vcp.tile([P, KT, HD], BF16, name="vcat_sa", tag="vcat",
                           bufs=2)
        vcat_ca = vcp.tile([P, KT, HD], BF16, name="vcat_ca", tag="vcat",
                           bufs=2)
        ycat_sa = ycatp.tile([P, HT, K], BF16, name="ycat_sa", tag="ycat")
        enc_xt = vcp.tile([P, MT, K], BF16, name="encT", tag="enct", bufs=1)

        with nc.named_scope("sa_att"):
            vload, vchunk = v_chunk_cbs(vcp, xt, "wv_sa", "bv_sa", vcat_sa)
            cvload, cvchunk = v_chunk_cbs(vcp, enc_xt, "wv_ca", "bv_ca",
                                          vcat_ca)
            pending = None
            for h in range(H):
                qh = qk_head_proj(vcp, attp, xt, "wq_sa", "bq_sa", h, "qh",
                                  prew=prew_q0 if h == 0 else None)
                kh = qk_head_proj(vcp, attp, xt, "wk_sa", "bk_sa", h, "kh",
                                  prew=prew_k0 if h == 0 else None)
                if h == 0:
                    vload[0]()
                    for cb in vchunk[:8]:
                        cb()
                elif h == 2:
                    vload[1]()
                elif h == 4:
                    for cb in vchunk[8:]:
                        cb()
                elif h == 5:
                    nc.sync.dma_start(
                        out=enc_xt,
                        in_=dram["xt_enc"].rearrange("p (mt k) -> p mt k",
                                                     mt=MT))
                elif h == 6:
                    cvload[0]()
                elif h == 7:
                    wots_sa = load_wo("wo_sa")
                    cvload[1]()
                pending = sda(attp, h, qh, kh, vcat_sa, ycat_sa, pending)
            pending()

        # ================= SA wo+ln (fillers: ca V-proj) =================
        x1t = new_xt("x1T")
        with nc.named_scope("sa_wo_ln"):
            wo_ln_block(ycat_sa, wots_sa, "bo_sa", dram["dec_nat"],
                        "g1", "b1", x1_store, x1t, filler=cvchunk,
                        fill_per_kt=2)

        # ================= CA attention =================
        ycat_ca = ycatp.tile([P, HT, K], BF16, name="ycat_ca", tag="ycat")
        with nc.named_scope("ca_att"):
            pending = None
            for h in range(H):
                qh = qk_head_proj(vcp, attp, x1t, "wq_ca", "bq_ca", h, "qh")
                kh = qk_head_proj(vcp, attp, enc_xt, "wk_ca", "bk_ca", h,
                                  "kh")
                if h == 7:
                    wots_ca = load_wo("wo_ca")
                pending = sda(attp, h, qh, kh, vcat_ca, ycat_ca, pending)
            pending()
        vc_ctx.__exit__(None, None, None)
        att_ctx.__exit__(None, None, None)

        # ================= CA wo+ln + FFN =================
        with tc.tile_pool(name="ffp", bufs=1) as ffp:
            x2t = new_xt("x2T")

            def w1_chunk_cb(kqh, ft, src_xt, rt):
                def chunk():
                    w1t = ffp.tile([P, MT, P], BF16, name=f"w1_{kqh}_{ft}",
                                   tag="w1t", bufs=6)
                    nc.sync.dma_start(
                        out=w1t,
                        in_=dram["w1"][ft].rearrange("p (mt d) -> p mt d",
                                                     mt=MT))
                    pf = ps.tile([P, 512], F32, name=f"pf{kqh}_{ft}",
                                 tag="ps")
                    for mt in range(MT):
                        nc.tensor.matmul(
                            pf, w1t[:, mt, :],
                            src_xt[:, mt, kqh * 512:(kqh + 1) * 512],
                            start=(mt == 0), stop=(mt == MT - 1))
                    if flags["bf1"]:
                        nc.scalar.activation(
                            rt[:, ft, :], pf, AF.Relu,
                            bias=bias_tiles["bf1"][:, ft:ft + 1])
                    else:
                        nc.scalar.activation(rt[:, ft, :], pf, AF.Relu)
                return chunk

            rt0 = ffp.tile([P, FT, 512], BF16, name="rt0", tag="rt", bufs=1)
            w1_fill = [w1_chunk_cb(0, ft, x2t, rt0) for ft in range(10)]

            with nc.named_scope("ca_wo_ln"):
                wo_ln_block(ycat_ca, wots_ca, "bo_ca", x1_store, "g2", "b2",
                            x2_store, x2t, filler=w1_fill, fill_per_kt=2,
                            fill_start=5)

            with nc.named_scope("ffn"):
                FTH = 8  # ft per w2 sub-slot (4 sub-slots per mh half)

                def w2_load(kqh, mh, sub):
                    s = ffp.tile([P, FTH, 512], BF16,
                                 name=f"w2_{kqh}_{mh}_{sub}", tag="w2s",
                                 bufs=5)
                    for i in range(FTH):
                        ft = sub * FTH + i
                        nc.sync.dma_start(
                            out=s[:, i, :],
                            in_=dram["w2"][ft, :, mh * 512:(mh + 1) * 512])
                    return s

                cur = [None] * 4
                for ft in range(10, FT):
                    w1_chunk_cb(0, ft, x2t, rt0)()
                    if ft in (12, 17, 22, 27):
                        cur[(ft - 12) // 5] = w2_load(0, 0, (ft - 12) // 5)

                def w2_half(kqh, rt):
                    """W2 for k-half kqh: per mh, per ks four 8-chains
                    accumulating into one psum bank; z3 in-place into the
                    x2 residual tile; LN per tile after mh1."""
                    nonlocal cur
                    x2r = []
                    stats_t = {}
                    for ks in range(4):
                        kt = kqh * 4 + ks
                        r = ffp.tile([P, M], BF16, name=f"x2r{kt}", tag="x2r",
                                     bufs=4)
                        nc.sync.dma_start(
                            out=r, in_=x2_store[kt * P:(kt + 1) * P, :])
                        x2r.append(r)
                    for mh in range(2):
                        subs = cur
                        if mh == 0:
                            cur = [None] * 4
                        for ks in range(4):
                            if mh == 0:
                                cur[ks] = w2_load(kqh, 1, ks)
                            pacc = ps.tile([P, 512], F32,
                                           name=f"po{kqh}_{mh}_{ks}",
                                           tag="ps")
                            for sub in range(4):
                                for i in range(FTH):
                                    nc.tensor.matmul(
                                        pacc,
                                        rt[:, sub * FTH + i,
                                           ks * P:(ks + 1) * P],
                                        subs[sub][:, i, :],
                                        start=(sub == 0 and i == 0),
                                        stop=(sub == 3 and i == FTH - 1))
                            sl = slice(mh * 512, (mh + 1) * 512)
                            if flags["bf2"]:
                                zt = natp.tile([P, 512], F32,
                                               name=f"zb{kqh}_{mh}_{ks}",
                                               tag="z3s")
                                nc.vector.scalar_tensor_tensor(
                                    out=zt, in0=pacc, scalar=1.0,
                                    in1=bias_tiles["bf2"][:, sl],
                                    op0=OP.bypass, op1=OP.add)
                                nc.vector.tensor_add(x2r[ks][:, sl], zt,
                                                     x2r[ks][:, sl])
                            else:
                                nc.vector.tensor_add(x2r[ks][:, sl], pacc,
                                                     x2r[ks][:, sl])
                            if mh == 0:
                                st = statp.tile([P, 2, 6], F32,
                                                name=f"stw{kqh}_{ks}",
                                                tag="stats")
                                nc.vector.bn_stats(out=st[:, 0, :],
                                                   in_=x2r[ks][:, 0:512])
                                stats_t[ks] = st
                            else:
                                ln_tail(x2r[ks], kqh * 4 + ks, "g3", "b3",
                                        None, True, False,
                                        stats=stats_t[ks], done_sg=1)

                w2_half(0, rt0)
                rt1 = ffp.tile([P, FT, 512], BF16, name="rt1", tag="rt",
                               bufs=1)
                cur = [None] * 4
                for ft in range(FT):
                    w1_chunk_cb(1, ft, x2t, rt1)()
                    if ft in (2, 7, 12, 17):
                        cur[(ft - 2) // 5] = w2_load(1, 0, (ft - 2) // 5)
                w2_half(1, rt1)


def _pack_inputs(inputs: dict):
    f32 = np.float32
    dec = np.asarray(inputs["dec"], f32)
    enc = np.asarray(inputs["enc"], f32)

    def nz(x):
        return bool(np.any(np.asarray(x) != 0.0))

    flags = {
        "bq_sa": nz(inputs["bq_sa"]), "bk_sa": nz(inputs["bk_sa"]),
        "bv_sa": nz(inputs["bv_sa"]), "bo_sa": nz(inputs["bo_sa"]),
        "bq_ca": nz(inputs["bq_ca"]), "bk_ca": nz(inputs["bk_ca"]),
        "bv_ca": nz(inputs["bv_ca"]), "bo_ca": nz(inputs["bo_ca"]),
        "bf1": nz(inputs["bf1"]), "bf2": nz(inputs["bf2"]),
        "g1": bool(np.any(np.asarray(inputs["g1"]) != 1.0)),
        "b1": nz(inputs["b1"]),
        "g2": bool(np.any(np.asarray(inputs["g2"]) != 1.0)),
        "b2": nz(inputs["b2"]),
        "g3": bool(np.any(np.asarray(inputs["g3"]) != 1.0)),
        "b3": nz(inputs["b3"]),
    }

    def bf(x):
        return np.ascontiguousarray(x).astype(BF)

    def qk_pack(w):
        w = np.asarray(w, f32)  # (H, DH, M)
        return bf(
            w.transpose(0, 2, 1).reshape(H, MT, P, DH).transpose(0, 2, 1, 3)
            .reshape(H, P, MT * DH))

    def v_pack(w):
        w = np.asarray(w, f32)  # (H, DH, M) -> WvT [m, hd]
        wt_ = w.transpose(2, 0, 1).reshape(M, HD)
        return bf(wt_.reshape(MT, P, HD))

    def o_pack(w):  # (M, HD) -> WoT (HD, M) -> (HT, P, M)
        return bf(np.ascontiguousarray(np.asarray(w, f32).T).reshape(
            HT, P, M))

    W1 = np.asarray(inputs["W1"], f32)
    W2 = np.asarray(inputs["W2"], f32)
    shared = {
        "wq_sa": qk_pack(inputs["Wq_sa"]), "wk_sa": qk_pack(inputs["Wk_sa"]),
        "wv_sa": v_pack(inputs["Wv_sa"]), "wo_sa": o_pack(inputs["Wo_sa"]),
        "wq_ca": qk_pack(inputs["Wq_ca"]), "wk_ca": qk_pack(inputs["Wk_ca"]),
        "wv_ca": v_pack(inputs["Wv_ca"]), "wo_ca": o_pack(inputs["Wo_ca"]),
        "w1": bf(W1.reshape(FT, P, MT, P).transpose(0, 3, 2, 1)
                 .reshape(FT, P, MT * P)),
        "w2": bf(np.ascontiguousarray(W2.T).reshape(FT, P, M)),
    }
    for nm in ("bq_sa", "bk_sa", "bq_ca", "bk_ca"):
        if flags[nm]:
            shared[nm] = np.ascontiguousarray(np.asarray(inputs[nm], f32).T)
    for nm in ("bv_sa", "bv_ca"):
        if flags[nm]:
            shared[nm] = np.asarray(inputs[nm], f32).reshape(HD)
    for nm in ("bo_sa", "bo_ca", "bf2", "g1", "b1", "g2", "b2", "g3", "b3"):
        if flags[nm]:
            shared[nm] = np.asarray(inputs[nm], f32)
    if flags["bf1"]:
        shared["bf1"] = np.ascontiguousarray(
            np.asarray(inputs["bf1"], f32).reshape(FT, P).T)

    def xt_pack(x):  # (K, M) -> transposed, partition-contiguous (P, MT*K)
        return bf(x.T.reshape(MT, P, K).transpose(1, 0, 2).reshape(P, MT * K))

    in_maps = []
    for c in range(N_CORES):
        m = dict(shared)
        m["xt_dec"] = xt_pack(dec[c])
        m["xt_enc"] = xt_pack(enc[c])
        m["dec_nat"] = bf(dec[c])
        in_maps.append(m)
    return flags, in_maps


_NC_CACHE: dict = {}


def kernel(**inputs) -> np.ndarray:
    flags, in_maps = _pack_inputs(inputs)
    key = tuple(sorted(flags.items()))
    if key not in _NC_CACHE:
        _NC_CACHE[key] = build_kernel(flags)
    nc = _NC_CACHE[key]
    res = run_bass_kernel_spmd(nc, in_maps, core_ids=list(range(N_CORES)))
    return np.stack([res.results[c]["out"] for c in range(N_CORES)])
